# revision 1
# baseline (speedup 1.0000x reference)
"""GCN 2-layer message-passing block on 8 Trainium2 NeuronCores — v3.

Same collapsed algebra as v2 (see kernel.py docstring), but the tab1
intermediate is produced directly in stage-B *token order*: each stage-B
token slot (tile, block, partition) owns one tab1 row, so stage B reads
tab1 with plain sequential DMAs (12-15KB descriptors) instead of a
row-granular dma_gather (768B descriptors).  Sources used by k stage-B
tokens are aggregated k times in stage A (~15% extra stage-A tokens).

Program uniformity across the 8 SPMD cores: each core sorts its stage-B
tiles by stage-A workload; the per-column sub-block count profile is the
column-wise max over cores (sorted profiles concentrate, so padding is
small).  Stage-A token streams are loaded in fixed 16-sub-block windows.
"""
import sys

sys.path.insert(0, "/opt/trn_rl_repo")

import numpy as np
import ml_dtypes

BF16 = ml_dtypes.bfloat16
FP8 = ml_dtypes.float8_e4m3

N_NODES = 100000
N_EDGES = 200000
H = 384
KB = H // 128
M_CORES = 8
NPC = N_NODES // M_CORES
B = 2                 # token blocks per dest tile (bin-packed)
CAP = B * 128         # 256 token slots per tile
SG = 4                # stage-B tiles per load/store group
WCAP = 496            # stage-A weight cap per stage-B tile
WG = 2 * SG           # stage-A columns per tab1 write group
LWIN = 16             # stage-A sub-blocks per load window


def _ffdpack(sizes):
    """Bin-pack dests into tiles of <=128 dests and <=CAP tokens (largest
    first).  Returns (tile_of, slot_of, tok_off, ntiles)."""
    n = sizes.size
    nb = max(int(np.ceil(sizes.sum() / (CAP - 2))), int(np.ceil(n / 128)))
    while True:
        rem_tok = np.full(nb, CAP, np.int64)
        rem_cnt = np.full(nb, 128, np.int64)
        tile_of = np.full(n, -1, np.int64)
        ok = True
        for s in range(int(sizes.max()), -1, -1):
            items = np.nonzero(sizes == s)[0]
            ii = 0
            while ii < items.size:
                elig = np.nonzero((rem_tok >= s) & (rem_cnt > 0))[0]
                if elig.size == 0:
                    ok = False
                    break
                elig = elig[np.argsort(-rem_tok[elig], kind="stable")]
                take = min(items.size - ii, elig.size)
                sel = elig[:take]
                tile_of[items[ii:ii + take]] = sel
                rem_tok[sel] -= s
                rem_cnt[sel] -= 1
                ii += take
            if not ok:
                break
        if ok:
            break
        nb += 1
    order = np.lexsort((np.arange(n), tile_of))
    slot_of = np.empty(n, np.int64)
    tok_off = np.empty(n, np.int64)
    cnt = np.zeros(nb, np.int64)
    tok = np.zeros(nb, np.int64)
    for i in order:
        t = tile_of[i]
        slot_of[i] = cnt[t]
        tok_off[i] = tok[t]
        cnt[t] += 1
        tok[t] += sizes[i]
    return tile_of, slot_of, tok_off, nb


def _pm_groups(tokens, ntiles, width, dtype, Q):
    """[ntiles*CAP, width] -> group-major [ntiles//Q, 128, Q*B, width]."""
    t = tokens.reshape(ntiles // Q, Q, B, 128, width)
    return np.ascontiguousarray(t.transpose(0, 3, 1, 2, 4).reshape(
        ntiles // Q, 128, Q * B, width)).astype(dtype)


def _dualpack(szs, ws):
    """Best-fit-decreasing pack of dests into tiles with <=128 dests,
    <=CAP-2 tokens and <=WCAP stage-A weight.  Returns (tile_of, ntiles)."""
    n = szs.size
    nb = max(int(np.ceil(szs.sum() / (CAP - 2))),
             int(np.ceil(ws.sum() / WCAP)), int(np.ceil(n / 128))) + 1
    order = np.argsort(-(ws * 1000000 + szs), kind="stable")
    while True:
        rem_tok = np.full(nb, CAP - 2, np.int64)
        rem_w = np.full(nb, WCAP, np.int64)
        rem_cnt = np.full(nb, 128, np.int64)
        tile_of = np.full(n, -1, np.int64)
        ok = True
        for d in order:
            feas = np.nonzero((rem_tok >= szs[d]) & (rem_w >= ws[d])
                              & (rem_cnt > 0))[0]
            if feas.size == 0:
                ok = False
                break
            b = feas[np.argmax(rem_w[feas])]
            tile_of[d] = b
            rem_tok[b] -= szs[d]
            rem_w[b] -= ws[d]
            rem_cnt[b] -= 1
        if ok:
            return tile_of, nb
        nb += 1


def _core_meta(row, col, cc, indeg_col):
    """Stage-B packing + program ordering for one core (token-balanced)."""
    em = (col // NPC) == cc
    er, ec = row[em], col[em]
    dlo = ec - cc * NPC
    szB = np.bincount(dlo, minlength=NPC)
    wE = indeg_col[er]                      # stage-A weight per edge token
    wD = np.bincount(dlo, weights=wE.astype(np.float64),
                     minlength=NPC).astype(np.int64)
    tileB, ntB = _dualpack(szB, wD)

    # dest slot within tile (order by dest id)
    order = np.lexsort((np.arange(NPC), tileB))
    slotB_d = np.empty(NPC, np.int64)
    cnt = np.zeros(ntB, np.int64)
    for i in order:
        slotB_d[i] = cnt[tileB[i]]
        cnt[tileB[i]] += 1

    # per-edge tile, then greedy token->half assignment balancing weight
    etile = tileB[dlo]
    eorder = np.lexsort((-wE, etile))       # per tile, heavy tokens first
    tsrc = np.full(ntB * CAP, -1, np.int64)
    tdsl = np.full(ntB * CAP, -1, np.int64)
    wh = np.zeros((ntB, 2), np.int64)
    ch = np.zeros((ntB, 2), np.int64)
    for e in eorder:
        t = etile[e]
        h = 0 if (wh[t, 0] <= wh[t, 1] and ch[t, 0] < 128) or                  ch[t, 1] >= 128 else 1
        tau = t * CAP + h * 128 + ch[t, h]
        tsrc[tau] = er[e]
        tdsl[tau] = slotB_d[dlo[e]]
        wh[t, h] += wE[e]
        ch[t, h] += 1

    cnt_phys = wh.reshape(-1)
    wt_tile = wh.sum(1)
    tile_order = np.argsort(-wt_tile, kind="stable")  # prog slot m -> phys
    blk_first = np.argmax(wh, axis=1)
    return dict(ntB=ntB, tileB=tileB, slotB_d=slotB_d, tsrc=tsrc, tdsl=tdsl,
                cnt_phys=cnt_phys, tile_order=tile_order, blk_first=blk_first)


def _prep(x, edge_index, W1, b1, W2, b2):
    row = np.asarray(edge_index[0], dtype=np.int64)
    col = np.asarray(edge_index[1], dtype=np.int64)
    xf = np.asarray(x, dtype=np.float64)

    deg = np.bincount(row, minlength=N_NODES).astype(np.float64)
    dis = deg ** -0.5
    a = np.bincount(col, weights=dis[row], minlength=N_NODES)
    cvec = dis * a
    c2 = dis * np.bincount(col, weights=(dis * cvec)[row], minlength=N_NODES)
    W12 = np.asarray(W2, np.float64) @ np.asarray(W1, np.float64)
    vv = np.asarray(W2, np.float64) @ np.asarray(b1, np.float64)
    xt = (dis[:, None] * xf).astype(BF16)

    # CSR of in-edges keyed by destination (col): stage-A edge lists
    indeg_col = np.bincount(col, minlength=N_NODES)
    order_c = np.argsort(col, kind="stable")
    row_by_col = row[order_c]
    coff = np.zeros(N_NODES + 1, np.int64)
    np.cumsum(indeg_col, out=coff[1:])

    metas = [_core_meta(row, col, cc, indeg_col) for cc in range(M_CORES)]
    NTB = max(m["ntB"] for m in metas)
    NTB += (-NTB) % SG
    NCOL = NTB * 2

    # per-core per-program-column sub-block counts -> global max profile
    def prog_cnt(m):
        cnt = np.zeros(NCOL, np.int64)
        ntB = m["ntB"]
        cp = m["cnt_phys"].reshape(ntB, 2)
        bf = m["blk_first"]
        ordered = np.stack([cp[np.arange(ntB), bf],
                            cp[np.arange(ntB), 1 - bf]], axis=1)
        cnt[:2 * ntB] = ordered[m["tile_order"]].reshape(-1)
        return cnt

    BA = np.maximum(1, -(-np.stack([prog_cnt(m) for m in metas]).max(0) // 128))
    colbase = np.zeros(NCOL + 1, np.int64)
    np.cumsum(BA, out=colbase[1:])
    TOT = int(colbase[-1])
    TOT += (-TOT) % LWIN

    w12 = np.ascontiguousarray(
        W12.T.astype(BF16).reshape(KB, 128, H).transpose(1, 0, 2))
    vrow = vv.astype(BF16).reshape(1, H)
    b2r = np.asarray(b2, dtype=BF16).reshape(1, H)

    in_maps = []
    perms = []
    for cc, m in enumerate(metas):
        ntB = m["ntB"]
        # program tile slot of each phys tile / block order per tile
        m_of_phys = np.empty(ntB, np.int64)
        m_of_phys[m["tile_order"]] = np.arange(ntB)
        bf = m["blk_first"]

        # program token slot for each phys token slot
        phys = np.arange(ntB * CAP)
        ptile = phys // CAP
        pblk = (phys % CAP) // 128
        ppar = phys % 128
        # block position under reorder: 0 if pblk == bf[ptile] else 1
        bpos = (pblk != bf[ptile]).astype(np.int64)
        pcol = m_of_phys[ptile] * 2 + bpos
        # token entries in program order
        tsrc_p = np.full(NCOL * 128, -1, np.int64)
        tdsl_p = np.full(NCOL * 128, -1, np.int64)
        tsrc_p[pcol * 128 + ppar] = m["tsrc"]
        tdsl_p[pcol * 128 + ppar] = m["tdsl"]

        valid = tsrc_p >= 0
        srcs = np.maximum(tsrc_p, 0)
        cnts = np.where(valid, indeg_col[srcs], 0)

        # expand: one entry per stage-A token
        tot_tok = int(cnts.sum())
        ent = np.repeat(np.arange(NCOL * 128), cnts)   # program token entry
        ecol = ent // 128
        epar = ent % 128                                # dest slot in column
        estart = np.repeat(coff[srcs], cnts)
        erank = np.arange(tot_tok) - np.repeat(np.concatenate(
            ([0], np.cumsum(cnts)))[:-1], cnts)
        erows = row_by_col[estart + erank]
        # position within the column's token stream
        ccnt = cnts.reshape(NCOL, 128).sum(1)
        cstart = np.zeros(NCOL + 1, np.int64)
        np.cumsum(ccnt, out=cstart[1:])
        k = np.arange(tot_tok) - np.repeat(cstart[:-1], ccnt)
        sub_abs = colbase[ecol] + k // 128
        part = k % 128

        t1 = np.zeros((128, TOT, H), dtype=BF16)
        t1[part, sub_abs] = xt[erows]
        s1 = np.zeros((128, TOT, 128), dtype=FP8)
        s1[part, sub_abs, epar] = 1.0

        dis2 = np.zeros((128, NCOL), dtype=np.float32)
        dis2[np.arange(NCOL * 128) % 128, np.arange(NCOL * 128) // 128] = \
            np.where(valid, (dis[srcs] ** 2), 0.0).astype(np.float32)

        # stage-B one-hot in program token order
        s2 = np.zeros((NCOL * 128, 128), dtype=np.float32)
        vi = np.nonzero(valid)[0]
        s2[vi, tdsl_p[vi]] = 1.0

        # owned-node indices in program tile numbering
        mt = m_of_phys[m["tileB"]]
        pown = mt * 128 + m["slotB_d"]
        pout = m["slotB_d"] * NTB + mt

        disc = np.zeros((128, NTB), dtype=np.float32)
        dd = dis[cc * NPC:(cc + 1) * NPC]
        disc[m["slotB_d"], mt] = dd.astype(np.float32)
        c12 = np.zeros((2, NTB * 128), dtype=BF16)
        c12[0, pown] = (c2[cc * NPC:(cc + 1) * NPC] / dd).astype(BF16)
        c12[1, pown] = (cvec[cc * NPC:(cc + 1) * NPC] / dd).astype(BF16)

        in_maps.append({
            "t1": t1, "s1": s1, "dis2": dis2,
            "s2": _pm_groups(s2, NTB, 128, FP8, SG),
            "disc": disc, "c12": c12,
            "w12": w12, "vb2": np.concatenate([vrow, b2r], axis=0),
        })
        perms.append(pout)
    return in_maps, dict(NTB=NTB, BA=tuple(int(b) for b in BA), TOT=TOT), perms


def _build(dims):
    import concourse.bass as bass
    import concourse.bacc as bacc
    import concourse.mybir as mybir
    import concourse.tile as tile

    dt = mybir.dt
    AF = mybir.ActivationFunctionType
    NTB, BA, TOT = dims["NTB"], dims["BA"], dims["TOT"]
    NCOL = NTB * 2
    ND = NTB * 128
    colbase = np.zeros(NCOL + 1, np.int64)
    np.cumsum(BA, out=colbase[1:])
    NW = TOT // LWIN

    nc = bacc.Bacc(None, target_bir_lowering=False)
    t1 = nc.dram_tensor("t1", [128, TOT, H], dt.bfloat16, kind="ExternalInput")
    s1 = nc.dram_tensor("s1", [128, TOT, 128], dt.float8e4, kind="ExternalInput")
    dis2 = nc.dram_tensor("dis2", [128, NCOL], dt.float32, kind="ExternalInput")
    s2 = nc.dram_tensor("s2", [NTB // SG, 128, SG * B, 128], dt.float8e4, kind="ExternalInput")
    disc = nc.dram_tensor("disc", [128, NTB], dt.float32, kind="ExternalInput")
    c12 = nc.dram_tensor("c12", [2, ND], dt.bfloat16, kind="ExternalInput")
    w12 = nc.dram_tensor("w12", [128, KB, H], dt.bfloat16, kind="ExternalInput")
    vb2 = nc.dram_tensor("vb2", [2, H], dt.bfloat16, kind="ExternalInput")
    NG = NTB // SG
    tabq = [nc.dram_tensor("tab1_%d" % i, [128, WG, H], dt.bfloat16,
                           kind="Internal") for i in range(NG)]
    out = nc.dram_tensor("out", [128, NTB, H], dt.bfloat16, kind="ExternalOutput")

    with tile.TileContext(nc) as tc:
        with (
            tc.tile_pool(name="const", bufs=1) as cp,
            tc.tile_pool(name="io", bufs=3) as iop,
            tc.tile_pool(name="stg", bufs=2) as stgp,
            tc.tile_pool(name="ps", bufs=2, space="PSUM") as psp,
        ):
            w12_sb = cp.tile([128, KB, H], dt.bfloat16)
            nc.sync.dma_start(w12_sb[:], w12[:])
            vb2_sb = cp.tile([2, H], dt.bfloat16)
            nc.sync.dma_start(vb2_sb[:], vb2[:])
            dis2_sb = cp.tile([128, NCOL], dt.float32)
            nc.sync.dma_start(dis2_sb[:], dis2[:])
            disc_sb = cp.tile([128, NTB], dt.float32)
            nc.sync.dma_start(disc_sb[:], disc[:])

            # ---------- interleaved stage A / stage B pipeline ----------
            wins = {}

            def ensure(lw):
                while ensure.hi < min(lw + 1, NW - 1) or ensure.hi < lw:
                    n = ensure.hi + 1
                    tt = iop.tile([128, LWIN, H], dt.bfloat16, tag="t1")
                    nc.sync.dma_start(tt[:], t1[:, n * LWIN:(n + 1) * LWIN, :])
                    ss = iop.tile([128, LWIN, 128], dt.float8e4, tag="s1")
                    nc.scalar.dma_start(ss[:], s1[:, n * LWIN:(n + 1) * LWIN, :])
                    wins[n] = (tt, ss)
                    wins.pop(n - 3, None)
                    ensure.hi = n
            ensure.hi = -1

            def emit_A(q):
                tws = stgp.tile([128, WG, H], dt.bfloat16, tag="tws")
                for h in range(WG):
                    c = q * WG + h
                    nsub = BA[c]
                    base = int(colbase[c])
                    psA = psp.tile([128, H], dt.float32, tag="psA", bufs=3)
                    for s in range(nsub):
                        ab = base + s
                        ensure(ab // LWIN)
                        tt, ss = wins[ab // LWIN]
                        off = ab % LWIN
                        nc.tensor.matmul(psA[:], ss[:, off, :], tt[:, off, :],
                                         start=(s == 0), stop=(s == nsub - 1))
                    if c % 2 == 0:
                        nc.vector.tensor_scalar_mul(
                            tws[:, h, :], psA[:], dis2_sb[:, c:c + 1])
                    else:
                        nc.scalar.activation(
                            tws[:, h, :], psA[:], AF.Copy,
                            scale=dis2_sb[:, c:c + 1])
                nc.gpsimd.dma_start(tabq[q][:], tws[:])

            def emit_B(q):
                s2_sb = iop.tile([128, SG * B, 128], dt.float8e4, tag="s2",
                                 bufs=2)
                nc.scalar.dma_start(s2_sb[:], s2[q])
                c12_sb = iop.tile([2, SG * 128], dt.bfloat16, tag="c12",
                                  bufs=2)
                nc.scalar.dma_start(c12_sb[:],
                                    c12[:, q * SG * 128:(q + 1) * SG * 128])
                g_sb = iop.tile([128, SG * B, H], dt.bfloat16, tag="g")
                nc.sync.dma_start(g_sb[:], tabq[q][:])
                ows = stgp.tile([128, SG, H], dt.bfloat16, tag="ows")
                for h in range(SG):
                    j = q * SG + h
                    psC = psp.tile([128, H], dt.float32, tag="psC", bufs=3)
                    for fs in range(KB):
                        for b in range(B):
                            nc.tensor.matmul(
                                psC[:, fs * 128:(fs + 1) * 128],
                                g_sb[:, h * B + b, fs * 128:(fs + 1) * 128],
                                s2_sb[:, h * B + b, :],
                                start=(b == 0), stop=(b == B - 1))
                    zf = iop.tile([128, H], dt.bfloat16, tag="zf")
                    if j % 2 == 0:
                        nc.vector.tensor_copy(zf[:], psC[:])
                    else:
                        nc.scalar.activation(zf[:], psC[:], AF.Copy)
                    psD = psp.tile([128, H], dt.float32, tag="psD")
                    for k in range(KB):
                        nc.tensor.matmul(psD[:], zf[:, k * 128:(k + 1) * 128],
                                         w12_sb[:, k, :],
                                         start=(k == 0), stop=False)
                    nc.tensor.matmul(psD[:], c12_sb[:, h * 128:(h + 1) * 128],
                                     vb2_sb[:], start=False, stop=True)
                    if j % 2 == 0:
                        nc.vector.tensor_scalar_mul(ows[:, h, :], psD[:],
                                                    disc_sb[:, j:j + 1])
                    else:
                        nc.scalar.activation(ows[:, h, :], psD[:], AF.Copy,
                                             scale=disc_sb[:, j:j + 1])
                nc.gpsimd.dma_start(out[:, q * SG:(q + 1) * SG, :], ows[:])

            for q in range(NG):
                emit_A(q)
                if q >= 1:
                    emit_B(q - 1)
            emit_B(NG - 1)
    nc.compile()
    return nc


_CACHE = {}


def _cache_key(dims):
    return (dims["NTB"], dims["TOT"], dims["BA"])


def kernel(x, edge_index, W1, b1, W2, b2):
    from concourse import bass_utils

    in_maps, dims, perms = _prep(x, edge_index, W1, b1, W2, b2)
    key = _cache_key(dims)
    if key not in _CACHE:
        _CACHE[key] = _build(dims)
    nc = _CACHE[key]
    res = bass_utils.run_bass_kernel_spmd(nc, in_maps, core_ids=list(range(M_CORES)))
    NTB = dims["NTB"]
    out = np.empty((N_NODES, H), np.float32)
    for cc in range(M_CORES):
        flat = np.asarray(res.results[cc]["out"]).reshape(128 * NTB, H)
        out[cc * NPC:(cc + 1) * NPC] = flat[perms[cc]].astype(np.float32)
    return out



# revision 6
# speedup vs baseline: 1.7253x; 1.7253x over previous
"""GCN 2-layer message-passing block on 8 Trainium2 NeuronCores — v4.

Collapsed algebra: z = (S^2 x) W12^T + c2 vv^T + cvec b2^T with
S = D^-1/2 A D^-1/2, W12 = W2 W1, vv = W2 b1.  The host expands the
two-hop pairs (v, w) with merged path weights sum_u dis[u]^2, pre-scales
each token row xt[w] = dis[w] x[w] by its pair weight, and lays tokens
out in sub-tiles of <=32 destinations and <=128 tokens.  The device does
a single one-hot aggregation (3 matmuls of 32 cols per sub-tile, K~126)
straight into the projection layout [feat-in-block, dest], then projects
with W12 per 128-dest group (4 sub-tiles) and adds the rank-2 bias
correction via a K=2 matmul.  One aggregation pass, no intermediate
DRAM tensors: ~53MB HBM traffic and ~200k PE columns per core.
"""
import sys

sys.path.insert(0, "/opt/trn_rl_repo")

import numpy as np
import ml_dtypes

BF16 = ml_dtypes.bfloat16
FP8 = ml_dtypes.float8_e4m3

N_NODES = 100000
N_EDGES = 200000
H = 384
KB = H // 128
M_CORES = 8
NPC = N_NODES // M_CORES
WD = 32               # dests per sub-tile (one-hot width)
ST = 4                # sub-tiles per group (group = 128 dests)
SG = 4                # groups per output DMA
LWIN = 16             # sub-blocks per DMA window (= 4 groups)


def _pack2(sizes, captok=128, capcnt=WD):
    """Greedy one-bin-at-a-time pack of (nonzero-size) dests into
    sub-tiles with <=captok tokens and <=capcnt dests: fill each bin with
    the largest remaining size that fits.  Returns (sub_of, nb)."""
    import heapq
    order = np.argsort(-sizes, kind="stable")
    nb = max(int(np.ceil(sizes.sum() / captok)), 1)
    heap = [(0, 0, b) for b in range(nb)]
    heapq.heapify(heap)
    sub_of = np.empty(sizes.size, np.int64)
    for i in order:
        s = int(sizes[i])
        parked = []
        while True:
            if not heap:
                tok, cnt, b = 0, 0, nb
                nb += 1
                break
            tok, cnt, b = heapq.heappop(heap)
            if tok + s <= captok and cnt < capcnt:
                break
            parked.append((tok, cnt, b))
        for p in parked:
            heapq.heappush(heap, p)
        sub_of[i] = b
        tok += s
        cnt += 1
        if tok < captok and cnt < capcnt:
            heapq.heappush(heap, (tok, cnt, b))
    return sub_of, nb


def _prep(x, edge_index, W1, b1, W2, b2):
    row = np.asarray(edge_index[0], dtype=np.int64)
    col = np.asarray(edge_index[1], dtype=np.int64)
    xf = np.asarray(x, dtype=np.float64)

    deg = np.bincount(row, minlength=N_NODES).astype(np.float64)
    dis = deg ** -0.5
    a = np.bincount(col, weights=dis[row], minlength=N_NODES)
    cvec = dis * a
    c2 = dis * np.bincount(col, weights=(dis * cvec)[row], minlength=N_NODES)
    W12 = np.asarray(W2, np.float64) @ np.asarray(W1, np.float64)
    vv = np.asarray(W2, np.float64) @ np.asarray(b1, np.float64)
    xt = (dis[:, None] * xf).astype(np.float32)

    # two-hop pairs: for each edge e=(u,v), expand in-edges (w,u)
    indeg = np.bincount(col, minlength=N_NODES)
    order_c = np.argsort(col, kind="stable")
    row_by_col = row[order_c]
    coff = np.zeros(N_NODES + 1, np.int64)
    np.cumsum(indeg, out=coff[1:])

    cnts = indeg[row]
    P = int(cnts.sum())
    csum = np.zeros(N_EDGES + 1, np.int64)
    np.cumsum(cnts, out=csum[1:])
    eidx = np.repeat(np.arange(N_EDGES), cnts)
    rank = np.arange(P) - np.repeat(csum[:-1], cnts)
    w_pair = row_by_col[np.repeat(coff[row], cnts) + rank]
    v_pair = col[eidx]
    wt_pair = (dis[row] ** 2)[eidx]
    key = v_pair * N_NODES + w_pair
    ukey, inv = np.unique(key, return_inverse=True)
    wts = np.bincount(inv, weights=wt_pair)
    v_m = ukey // N_NODES
    w_m = ukey % N_NODES

    w12 = np.ascontiguousarray(
        W12.T.astype(BF16).reshape(KB, 128, H).transpose(1, 0, 2))
    vb2 = np.stack([vv.astype(BF16),
                    np.asarray(b2, np.float64).astype(BF16)], axis=0)

    # per-core packing (shared program => shared NG)
    cores = []
    for cc in range(M_CORES):
        m = (v_m // NPC) == cc
        dlo = v_m[m] - cc * NPC
        wsrc = w_m[m]
        wt = wts[m]
        sizes = np.bincount(dlo, minlength=NPC)
        # pack nonzero dests token-tight, then fill count slots with zeros
        nz = np.nonzero(sizes > 0)[0]
        sub_nz, nb = _pack2(sizes[nz])
        sub_of = np.empty(NPC, np.int64)
        sub_of[nz] = sub_nz
        zr = np.nonzero(sizes == 0)[0]
        free = WD - np.bincount(sub_nz, minlength=nb)
        slots = np.repeat(np.arange(nb), free)
        if slots.size < zr.size:
            extra = -(-(zr.size - slots.size) // WD)
            slots = np.concatenate(
                [slots, np.repeat(np.arange(nb, nb + extra), WD)])
            nb += extra
        sub_of[zr] = slots[:zr.size]
        cores.append((dlo, wsrc, wt, sizes, sub_of, nb))
    NG = max(-(-c[5] // ST) for c in cores)
    NG += (-NG) % SG
    TOT = NG * ST

    in_maps = []
    perms = []
    for cc, (dlo, wsrc, wt, sizes, sub_of, nb) in enumerate(cores):
        # dest slot + token offset within sub-tile
        order = np.lexsort((np.arange(NPC), sub_of))
        dslot = np.empty(NPC, np.int64)
        tok_off = np.empty(NPC, np.int64)
        cnt = np.zeros(nb, np.int64)
        tok = np.zeros(nb, np.int64)
        for i in order:
            t = sub_of[i]
            dslot[i] = cnt[t]
            tok_off[i] = tok[t]
            cnt[t] += 1
            tok[t] += sizes[i]

        # pairs sorted by (dlo, w) already (ukey order)
        starts = np.zeros(NPC + 1, np.int64)
        np.cumsum(sizes, out=starts[1:])
        prank = np.arange(dlo.size) - starts[dlo]
        part = tok_off[dlo] + prank
        sb = sub_of[dlo]

        t2 = np.zeros((128, TOT, H), dtype=BF16)
        t2[part, sb] = (xt[wsrc] * wt[:, None].astype(np.float32))
        s1 = np.zeros((128, TOT, WD), dtype=FP8)
        s1[part, sb, dslot[dlo]] = 1.0

        g_d = sub_of // ST
        part_d = (sub_of % ST) * WD + dslot
        dd = dis[cc * NPC:(cc + 1) * NPC]
        disc = np.zeros((128, NG), dtype=np.float32)
        disc[part_d, g_d] = dd.astype(np.float32)
        c12 = np.zeros((2, NG * 128), dtype=BF16)
        pown = g_d * 128 + part_d
        c12[0, pown] = (c2[cc * NPC:(cc + 1) * NPC] / dd).astype(BF16)
        c12[1, pown] = (cvec[cc * NPC:(cc + 1) * NPC] / dd).astype(BF16)

        in_maps.append({
            "t2": t2, "s1": s1, "disc": disc, "c12": c12,
            "w12": w12, "vb2": vb2,
        })
        perms.append(part_d * NG + g_d)
    return in_maps, dict(NG=NG), perms


def _build(dims):
    import concourse.bass as bass
    import concourse.bacc as bacc
    import concourse.mybir as mybir
    import concourse.tile as tile

    dt = mybir.dt
    AF = mybir.ActivationFunctionType
    NG = dims["NG"]
    TOT = NG * ST
    NW = TOT // LWIN

    nc = bacc.Bacc(None, target_bir_lowering=False)
    t2 = nc.dram_tensor("t2", [128, TOT, H], dt.bfloat16, kind="ExternalInput")
    s1 = nc.dram_tensor("s1", [128, TOT, WD], dt.float8e4, kind="ExternalInput")
    disc = nc.dram_tensor("disc", [128, NG], dt.float32, kind="ExternalInput")
    c12 = nc.dram_tensor("c12", [2, NG * 128], dt.bfloat16, kind="ExternalInput")
    w12 = nc.dram_tensor("w12", [128, KB, H], dt.bfloat16, kind="ExternalInput")
    vb2 = nc.dram_tensor("vb2", [2, H], dt.bfloat16, kind="ExternalInput")
    out = nc.dram_tensor("out", [128, NG, H], dt.bfloat16, kind="ExternalOutput")

    with tile.TileContext(nc) as tc:
        with (
            tc.tile_pool(name="const", bufs=1) as cp,
            tc.tile_pool(name="io", bufs=3) as iop,
            tc.tile_pool(name="stg", bufs=2) as stgp,
            tc.tile_pool(name="ps", bufs=2, space="PSUM") as psp,
        ):
            w12_sb = cp.tile([128, KB, H], dt.bfloat16)
            nc.sync.dma_start(w12_sb[:], w12[:])
            vb2_sb = cp.tile([2, H], dt.bfloat16)
            nc.sync.dma_start(vb2_sb[:], vb2[:])
            disc_sb = cp.tile([128, NG], dt.float32)
            nc.sync.dma_start(disc_sb[:], disc[:])
            c12_sb = cp.tile([2, NG * 128], dt.bfloat16)
            nc.sync.dma_start(c12_sb[:], c12[:])

            wins = {}

            def ensure(wi):
                while ensure.hi < min(wi + 1, NW - 1) or ensure.hi < wi:
                    nx = ensure.hi + 1
                    tt = iop.tile([128, LWIN, H], dt.bfloat16, tag="t2")
                    nc.sync.dma_start(tt[:], t2[:, nx * LWIN:(nx + 1) * LWIN, :])
                    ss = iop.tile([128, LWIN, WD], dt.float8e4, tag="s1")
                    nc.scalar.dma_start(ss[:], s1[:, nx * LWIN:(nx + 1) * LWIN, :])
                    wins[nx] = (tt, ss)
                    wins.pop(nx - 3, None)
                    ensure.hi = nx
            ensure.hi = -1

            def emit_psC(j):
                wi = (j * ST) // LWIN
                ensure(wi)
                tt, ss = wins[wi]
                psC = psp.tile([128, H], dt.float32, tag="psC", bufs=3)
                for st in range(ST):
                    sb = (j * ST) % LWIN + st
                    for f in range(KB):
                        o = f * 128 + st * WD
                        nc.tensor.matmul(psC[:, o:o + WD],
                                         tt[:, sb, f * 128:(f + 1) * 128],
                                         ss[:, sb, :], start=True, stop=True)
                zf = iop.tile([128, H], dt.bfloat16, tag="zf", bufs=3)
                if j % 2 == 0:
                    nc.vector.tensor_copy(zf[:], psC[:])
                else:
                    nc.scalar.activation(zf[:], psC[:], AF.Copy)
                return zf

            def emit_psD(j, zf, ows):
                psD = psp.tile([128, H], dt.float32, tag="psD", bufs=3)
                for k in range(KB):
                    nc.tensor.matmul(psD[:], zf[:, k * 128:(k + 1) * 128],
                                     w12_sb[:, k, :], start=(k == 0), stop=False)
                nc.tensor.matmul(psD[:], c12_sb[:, j * 128:(j + 1) * 128],
                                 vb2_sb[:], start=False, stop=True)
                if j % 2 == 0:
                    nc.scalar.activation(ows[:, j % SG, :], psD[:], AF.Copy,
                                         scale=disc_sb[:, j:j + 1])
                else:
                    nc.vector.tensor_scalar_mul(ows[:, j % SG, :], psD[:],
                                                disc_sb[:, j:j + 1])

            prev = None
            ows = None
            for j in range(NG + 1):
                zf = emit_psC(j) if j < NG else None
                if prev is not None:
                    pj, pzf = prev
                    if pj % SG == 0:
                        ows = stgp.tile([128, SG, H], dt.bfloat16, tag="ows")
                    emit_psD(pj, pzf, ows)
                    if pj % SG == SG - 1:
                        nc.gpsimd.dma_start(
                            out[:, pj - SG + 1:pj + 1, :], ows[:])
                prev = (j, zf) if j < NG else None
    nc.compile()
    return nc


_CACHE = {}


def kernel(x, edge_index, W1, b1, W2, b2):
    from concourse import bass_utils

    in_maps, dims, perms = _prep(x, edge_index, W1, b1, W2, b2)
    key = dims["NG"]
    if key not in _CACHE:
        _CACHE[key] = _build(dims)
    nc = _CACHE[key]
    res = bass_utils.run_bass_kernel_spmd(nc, in_maps, core_ids=list(range(M_CORES)))
    NG = dims["NG"]
    out = np.empty((N_NODES, H), np.float32)
    for cc in range(M_CORES):
        flat = np.asarray(res.results[cc]["out"]).reshape(128 * NG, H)
        out[cc * NPC:(cc + 1) * NPC] = flat[perms[cc]].astype(np.float32)
    return out


# revision 13
# speedup vs baseline: 1.9746x; 1.1446x over previous
"""GCN 2-layer message-passing block on 8 Trainium2 NeuronCores — v4.

Collapsed algebra: z = (S^2 x) W12^T + c2 vv^T + cvec b2^T with
S = D^-1/2 A D^-1/2, W12 = W2 W1, vv = W2 b1.  The host expands the
two-hop pairs (v, w) with merged path weights sum_u dis[u]^2, pre-scales
each token row xt[w] = dis[w] x[w] by its pair weight, and lays tokens
out in sub-tiles of <=32 destinations and <=128 tokens.  The device does
a single one-hot aggregation (3 matmuls of 32 cols per sub-tile, K~126)
straight into the projection layout [feat-in-block, dest], then projects
with W12 per 128-dest group (4 sub-tiles) and adds the rank-2 bias
correction via a K=2 matmul.  One aggregation pass, no intermediate
DRAM tensors: ~53MB HBM traffic and ~200k PE columns per core.
"""
import sys

sys.path.insert(0, "/opt/trn_rl_repo")

import numpy as np
import ml_dtypes

BF16 = ml_dtypes.bfloat16
FP8 = ml_dtypes.float8_e4m3

N_NODES = 100000
N_EDGES = 200000
H = 384
KB = H // 128
M_CORES = 8
NPC = N_NODES // M_CORES
WD = 32               # dests per sub-tile (one-hot width)
ST = 4                # sub-tiles per group (group = 128 dests)
SG = 4                # groups per output DMA
LWIN = 32             # sub-blocks per DMA window (= 8 groups)


def _pack2(sizes, captok=128, capcnt=WD):
    """Greedy one-bin-at-a-time pack of (nonzero-size) dests into
    sub-tiles with <=captok tokens and <=capcnt dests: fill each bin with
    the largest remaining size that fits.  Returns (sub_of, nb)."""
    import heapq
    order = np.argsort(-sizes, kind="stable")
    nb = max(int(np.ceil(sizes.sum() / captok)), 1)
    heap = [(0, 0, b) for b in range(nb)]
    heapq.heapify(heap)
    sub_of = np.empty(sizes.size, np.int64)
    for i in order:
        s = int(sizes[i])
        parked = []
        while True:
            if not heap:
                tok, cnt, b = 0, 0, nb
                nb += 1
                break
            tok, cnt, b = heapq.heappop(heap)
            if tok + s <= captok and cnt < capcnt:
                break
            parked.append((tok, cnt, b))
        for p in parked:
            heapq.heappush(heap, p)
        sub_of[i] = b
        tok += s
        cnt += 1
        if tok < captok and cnt < capcnt:
            heapq.heappush(heap, (tok, cnt, b))
    return sub_of, nb


def _prep(x, edge_index, W1, b1, W2, b2):
    row = np.asarray(edge_index[0], dtype=np.int64)
    col = np.asarray(edge_index[1], dtype=np.int64)
    xf = np.asarray(x, dtype=np.float64)

    deg = np.bincount(row, minlength=N_NODES).astype(np.float64)
    dis = deg ** -0.5
    a = np.bincount(col, weights=dis[row], minlength=N_NODES)
    cvec = dis * a
    c2 = dis * np.bincount(col, weights=(dis * cvec)[row], minlength=N_NODES)
    W12 = np.asarray(W2, np.float64) @ np.asarray(W1, np.float64)
    vv = np.asarray(W2, np.float64) @ np.asarray(b1, np.float64)
    xt = (dis[:, None] * xf).astype(np.float32)

    # two-hop pairs: for each edge e=(u,v), expand in-edges (w,u)
    indeg = np.bincount(col, minlength=N_NODES)
    order_c = np.argsort(col, kind="stable")
    row_by_col = row[order_c]
    coff = np.zeros(N_NODES + 1, np.int64)
    np.cumsum(indeg, out=coff[1:])

    cnts = indeg[row]
    P = int(cnts.sum())
    csum = np.zeros(N_EDGES + 1, np.int64)
    np.cumsum(cnts, out=csum[1:])
    eidx = np.repeat(np.arange(N_EDGES), cnts)
    rank = np.arange(P) - np.repeat(csum[:-1], cnts)
    w_pair = row_by_col[np.repeat(coff[row], cnts) + rank]
    v_pair = col[eidx]
    wt_pair = (dis[row] ** 2)[eidx]
    key = v_pair * N_NODES + w_pair
    ukey, inv = np.unique(key, return_inverse=True)
    wts = np.bincount(inv, weights=wt_pair)
    v_m = ukey // N_NODES
    w_m = ukey % N_NODES

    w12 = np.ascontiguousarray(
        W12.T.astype(BF16).reshape(KB, 128, H).transpose(1, 0, 2))
    vb2 = np.stack([vv.astype(BF16),
                    np.asarray(b2, np.float64).astype(BF16)], axis=0)

    # per-core packing (shared program => shared NG)
    cores = []
    for cc in range(M_CORES):
        m = (v_m // NPC) == cc
        dlo = v_m[m] - cc * NPC
        wsrc = w_m[m]
        wt = wts[m]
        sizes = np.bincount(dlo, minlength=NPC)
        # pack nonzero dests token-tight, then fill count slots with zeros
        nz = np.nonzero(sizes > 0)[0]
        sub_nz, nb = _pack2(sizes[nz])
        sub_of = np.empty(NPC, np.int64)
        sub_of[nz] = sub_nz
        zr = np.nonzero(sizes == 0)[0]
        free = WD - np.bincount(sub_nz, minlength=nb)
        slots = np.repeat(np.arange(nb), free)
        if slots.size < zr.size:
            extra = -(-(zr.size - slots.size) // WD)
            slots = np.concatenate(
                [slots, np.repeat(np.arange(nb, nb + extra), WD)])
            nb += extra
        sub_of[zr] = slots[:zr.size]
        cores.append((dlo, wsrc, wt, sizes, sub_of, nb))
    NG = max(-(-c[5] // ST) for c in cores)
    NG += (-NG) % (LWIN // ST)
    TOT = NG * ST

    in_maps = []
    perms = []
    for cc, (dlo, wsrc, wt, sizes, sub_of, nb) in enumerate(cores):
        # dest slot + token offset within sub-tile
        order = np.lexsort((np.arange(NPC), sub_of))
        dslot = np.empty(NPC, np.int64)
        tok_off = np.empty(NPC, np.int64)
        cnt = np.zeros(nb, np.int64)
        tok = np.zeros(nb, np.int64)
        for i in order:
            t = sub_of[i]
            dslot[i] = cnt[t]
            tok_off[i] = tok[t]
            cnt[t] += 1
            tok[t] += sizes[i]

        # pairs sorted by (dlo, w) already (ukey order)
        starts = np.zeros(NPC + 1, np.int64)
        np.cumsum(sizes, out=starts[1:])
        prank = np.arange(dlo.size) - starts[dlo]
        part = tok_off[dlo] + prank
        sb = sub_of[dlo]

        t2 = np.zeros((128, TOT, H), dtype=BF16)
        t2[part, sb] = (xt[wsrc] * wt[:, None].astype(np.float32))
        s1 = np.zeros((128, TOT, WD), dtype=FP8)
        s1[part, sb, dslot[dlo]] = 1.0

        g_d = sub_of // ST
        part_d = (sub_of % ST) * WD + dslot
        dd = dis[cc * NPC:(cc + 1) * NPC]
        disc = np.zeros((128, NG), dtype=np.float32)
        disc[part_d, g_d] = dd.astype(np.float32)

        in_maps.append({
            "t2": t2, "s1": s1, "disc": disc, "w12": w12,
        })
        perms.append(part_d * NG + g_d)
    # x-independent rank-2 bias, added host-side after the gather
    corr = (np.outer(c2, vv) + np.outer(cvec, np.asarray(b2, np.float64))
            ).astype(np.float32)
    return in_maps, dict(NG=NG), perms, corr


def _build(dims):
    import concourse.bass as bass
    import concourse.bacc as bacc
    import concourse.mybir as mybir
    import concourse.tile as tile

    dt = mybir.dt
    AF = mybir.ActivationFunctionType
    NG = dims["NG"]
    TOT = NG * ST
    NW = TOT // LWIN

    nc = bacc.Bacc(None, target_bir_lowering=False)
    t2 = nc.dram_tensor("t2", [128, TOT, H], dt.bfloat16, kind="ExternalInput")
    s1 = nc.dram_tensor("s1", [128, TOT, WD], dt.float8e4, kind="ExternalInput")
    disc = nc.dram_tensor("disc", [128, NG], dt.float32, kind="ExternalInput")
    w12 = nc.dram_tensor("w12", [128, KB, H], dt.bfloat16, kind="ExternalInput")
    out = nc.dram_tensor("out", [128, NG, H], dt.bfloat16, kind="ExternalOutput")

    with tile.TileContext(nc) as tc:
        with (
            tc.tile_pool(name="const", bufs=1) as cp,
            tc.tile_pool(name="io", bufs=3) as iop,
            tc.tile_pool(name="stg", bufs=2) as stgp,
            tc.tile_pool(name="ps", bufs=2, space="PSUM") as psp,
        ):
            w12_sb = cp.tile([128, KB, H], dt.bfloat16)
            nc.sync.dma_start(w12_sb[:], w12[:])
            disc_sb = cp.tile([128, NG], dt.float32)
            nc.sync.dma_start(disc_sb[:], disc[:])

            wins = {}
            HW = LWIN // 2

            def ensure(wi):
                while ensure.hi < min(wi + 1, NW - 1) or ensure.hi < wi:
                    nx = ensure.hi + 1
                    tt = iop.tile([128, LWIN, H], dt.bfloat16, tag="t2")
                    nc.sync.dma_start(tt[:, :HW],
                                      t2[:, nx * LWIN:nx * LWIN + HW, :])
                    nc.scalar.dma_start(tt[:, HW:],
                                        t2[:, nx * LWIN + HW:(nx + 1) * LWIN, :])
                    ss = iop.tile([128, LWIN, WD], dt.float8e4, tag="s1")
                    nc.gpsimd.dma_start(ss[:], s1[:, nx * LWIN:(nx + 1) * LWIN, :])
                    wins[nx] = (tt, ss)
                    wins.pop(nx - 3, None)
                    ensure.hi = nx
            ensure.hi = -1

            def emit_psC(j):
                wi = (j * ST) // LWIN
                ensure(wi)
                tt, ss = wins[wi]
                psC = psp.tile([128, H], dt.float32, tag="psC", bufs=3)
                for st in range(ST):
                    sb = (j * ST) % LWIN + st
                    for f in range(KB):
                        o = f * 128 + st * WD
                        nc.tensor.matmul(psC[:, o:o + WD],
                                         tt[:, sb, f * 128:(f + 1) * 128],
                                         ss[:, sb, :], start=True, stop=True)
                zf = iop.tile([128, H], dt.bfloat16, tag="zf", bufs=3)
                if j % 2 == 0:
                    nc.vector.tensor_copy(zf[:], psC[:])
                else:
                    nc.scalar.activation(zf[:], psC[:], AF.Copy)
                return zf

            def emit_psD(j, zf, ows):
                psD = psp.tile([128, H], dt.float32, tag="psD", bufs=3)
                for k in range(KB):
                    nc.tensor.matmul(psD[:], zf[:, k * 128:(k + 1) * 128],
                                     w12_sb[:, k, :], start=(k == 0),
                                     stop=(k == KB - 1))
                if j % 2 == 0:
                    nc.scalar.activation(ows[:, j % SG, :], psD[:], AF.Copy,
                                         scale=disc_sb[:, j:j + 1])
                else:
                    nc.vector.tensor_scalar_mul(ows[:, j % SG, :], psD[:],
                                                disc_sb[:, j:j + 1])

            prev = None
            ows = None
            for j in range(NG + 1):
                zf = emit_psC(j) if j < NG else None
                if prev is not None:
                    pj, pzf = prev
                    if pj % SG == 0:
                        ows = stgp.tile([128, SG, H], dt.bfloat16, tag="ows")
                    emit_psD(pj, pzf, ows)
                    if pj % SG == SG - 1:
                        nc.gpsimd.dma_start(
                            out[:, pj - SG + 1:pj + 1, :], ows[:])
                prev = (j, zf) if j < NG else None
    nc.compile()
    return nc


_CACHE = {}


def kernel(x, edge_index, W1, b1, W2, b2):
    from concourse import bass_utils

    in_maps, dims, perms, corr = _prep(x, edge_index, W1, b1, W2, b2)
    key = dims["NG"]
    if key not in _CACHE:
        _CACHE[key] = _build(dims)
    nc = _CACHE[key]
    res = bass_utils.run_bass_kernel_spmd(nc, in_maps, core_ids=list(range(M_CORES)))
    NG = dims["NG"]
    out = np.empty((N_NODES, H), np.float32)
    for cc in range(M_CORES):
        flat = np.asarray(res.results[cc]["out"]).reshape(128 * NG, H)
        out[cc * NPC:(cc + 1) * NPC] = flat[perms[cc]].astype(np.float32)
    out += corr
    return out


# revision 17
# speedup vs baseline: 2.1152x; 1.0712x over previous
"""GCN 2-layer message-passing block on 8 Trainium2 NeuronCores — v4.

Collapsed algebra: z = (S^2 x) W12^T + c2 vv^T + cvec b2^T with
S = D^-1/2 A D^-1/2, W12 = W2 W1, vv = W2 b1.  The host expands the
two-hop pairs (v, w) with merged path weights sum_u dis[u]^2, pre-scales
each token row xt[w] = dis[w] x[w] by its pair weight, and lays tokens
out in sub-tiles of <=32 destinations and <=128 tokens.  The device does
a single one-hot aggregation (3 matmuls of 32 cols per sub-tile, K~126)
straight into the projection layout [feat-in-block, dest], then projects
with W12 per 128-dest group (4 sub-tiles) and adds the rank-2 bias
correction via a K=2 matmul.  One aggregation pass, no intermediate
DRAM tensors: ~53MB HBM traffic and ~200k PE columns per core.
"""
import sys

sys.path.insert(0, "/opt/trn_rl_repo")

import numpy as np
import ml_dtypes

BF16 = ml_dtypes.bfloat16
FP8 = ml_dtypes.float8_e4m3

N_NODES = 100000
N_EDGES = 200000
H = 384
KB = H // 128
M_CORES = 8
NPC = N_NODES // M_CORES
WD = 32               # dests per sub-tile (one-hot width)
ST = 4                # sub-tiles per group (group = 128 dests)
SG = 4                # groups per output DMA
LWIN = 16             # sub-blocks per DMA window (= 4 groups)


def _pack2(sizes, captok=128, capcnt=WD):
    """Greedy one-bin-at-a-time pack of (nonzero-size) dests into
    sub-tiles with <=captok tokens and <=capcnt dests: fill each bin with
    the largest remaining size that fits.  Returns (sub_of, nb)."""
    import heapq
    order = np.argsort(-sizes, kind="stable")
    nb = max(int(np.ceil(sizes.sum() / captok)), 1)
    heap = [(0, 0, b) for b in range(nb)]
    heapq.heapify(heap)
    sub_of = np.empty(sizes.size, np.int64)
    for i in order:
        s = int(sizes[i])
        parked = []
        while True:
            if not heap:
                tok, cnt, b = 0, 0, nb
                nb += 1
                break
            tok, cnt, b = heapq.heappop(heap)
            if tok + s <= captok and cnt < capcnt:
                break
            parked.append((tok, cnt, b))
        for p in parked:
            heapq.heappush(heap, p)
        sub_of[i] = b
        tok += s
        cnt += 1
        if tok < captok and cnt < capcnt:
            heapq.heappush(heap, (tok, cnt, b))
    return sub_of, nb


def _prep(x, edge_index, W1, b1, W2, b2):
    row = np.asarray(edge_index[0], dtype=np.int64)
    col = np.asarray(edge_index[1], dtype=np.int64)
    xf = np.asarray(x, dtype=np.float64)

    deg = np.bincount(row, minlength=N_NODES).astype(np.float64)
    dis = deg ** -0.5
    a = np.bincount(col, weights=dis[row], minlength=N_NODES)
    cvec = dis * a
    c2 = dis * np.bincount(col, weights=(dis * cvec)[row], minlength=N_NODES)
    W12 = np.asarray(W2, np.float64) @ np.asarray(W1, np.float64)
    vv = np.asarray(W2, np.float64) @ np.asarray(b1, np.float64)
    xt = (dis[:, None] * xf).astype(np.float32)

    # two-hop pairs: for each edge e=(u,v), expand in-edges (w,u)
    indeg = np.bincount(col, minlength=N_NODES)
    order_c = np.argsort(col, kind="stable")
    row_by_col = row[order_c]
    coff = np.zeros(N_NODES + 1, np.int64)
    np.cumsum(indeg, out=coff[1:])

    cnts = indeg[row]
    P = int(cnts.sum())
    csum = np.zeros(N_EDGES + 1, np.int64)
    np.cumsum(cnts, out=csum[1:])
    eidx = np.repeat(np.arange(N_EDGES), cnts)
    rank = np.arange(P) - np.repeat(csum[:-1], cnts)
    w_pair = row_by_col[np.repeat(coff[row], cnts) + rank]
    v_pair = col[eidx]
    wt_pair = (dis[row] ** 2)[eidx]
    key = v_pair * N_NODES + w_pair
    ukey, inv = np.unique(key, return_inverse=True)
    wts = np.bincount(inv, weights=wt_pair)
    v_m = ukey // N_NODES
    w_m = ukey % N_NODES

    w12 = np.ascontiguousarray(
        W12.T.astype(BF16).reshape(KB, 128, H).transpose(1, 0, 2))
    vb2 = np.stack([vv.astype(BF16),
                    np.asarray(b2, np.float64).astype(BF16)], axis=0)

    # per-core packing (shared program => shared NG)
    cores = []
    for cc in range(M_CORES):
        m = (v_m // NPC) == cc
        dlo = v_m[m] - cc * NPC
        wsrc = w_m[m]
        wt = wts[m]
        sizes = np.bincount(dlo, minlength=NPC)
        # pack nonzero dests token-tight, then fill count slots with zeros
        nz = np.nonzero(sizes > 0)[0]
        sub_nz, nb = _pack2(sizes[nz])
        sub_of = np.empty(NPC, np.int64)
        sub_of[nz] = sub_nz
        zr = np.nonzero(sizes == 0)[0]
        free = WD - np.bincount(sub_nz, minlength=nb)
        slots = np.repeat(np.arange(nb), free)
        if slots.size < zr.size:
            extra = -(-(zr.size - slots.size) // WD)
            slots = np.concatenate(
                [slots, np.repeat(np.arange(nb, nb + extra), WD)])
            nb += extra
        sub_of[zr] = slots[:zr.size]
        cores.append((dlo, wsrc, wt, sizes, sub_of, nb))
    NG = max(-(-c[5] // ST) for c in cores)
    NG += (-NG) % (LWIN // ST)
    TOT = NG * ST

    in_maps = []
    perms = []
    for cc, (dlo, wsrc, wt, sizes, sub_of, nb) in enumerate(cores):
        # dest slot + token offset within sub-tile
        order = np.lexsort((np.arange(NPC), sub_of))
        dslot = np.empty(NPC, np.int64)
        tok_off = np.empty(NPC, np.int64)
        cnt = np.zeros(nb, np.int64)
        tok = np.zeros(nb, np.int64)
        for i in order:
            t = sub_of[i]
            dslot[i] = cnt[t]
            tok_off[i] = tok[t]
            cnt[t] += 1
            tok[t] += sizes[i]

        # pairs sorted by (dlo, w) already (ukey order)
        starts = np.zeros(NPC + 1, np.int64)
        np.cumsum(sizes, out=starts[1:])
        prank = np.arange(dlo.size) - starts[dlo]
        part = tok_off[dlo] + prank
        sb = sub_of[dlo]

        t2 = np.zeros((128, TOT, H), dtype=BF16)
        t2[part, sb] = (xt[wsrc] * wt[:, None].astype(np.float32))
        s1 = np.zeros((128, TOT, WD), dtype=FP8)
        s1[part, sb, dslot[dlo]] = 1.0

        g_d = sub_of // ST
        part_d = (sub_of % ST) * WD + dslot
        dd = dis[cc * NPC:(cc + 1) * NPC]
        disc = np.zeros((128, NG), dtype=np.float32)
        disc[part_d, g_d] = dd.astype(np.float32)

        in_maps.append({
            "t2": t2, "s1": s1, "disc": disc, "w12": w12,
        })
        perms.append(part_d * NG + g_d)
    # x-independent rank-2 bias, added host-side after the gather
    corr = (np.outer(c2, vv) + np.outer(cvec, np.asarray(b2, np.float64))
            ).astype(np.float32)
    return in_maps, dict(NG=NG), perms, corr


def _build(dims):
    import concourse.bass as bass
    import concourse.bacc as bacc
    import concourse.mybir as mybir
    import concourse.tile as tile

    dt = mybir.dt
    AF = mybir.ActivationFunctionType
    NG = dims["NG"]
    TOT = NG * ST
    NW = TOT // LWIN

    nc = bacc.Bacc(None, target_bir_lowering=False)
    t2 = nc.dram_tensor("t2", [128, TOT, H], dt.bfloat16, kind="ExternalInput")
    s1 = nc.dram_tensor("s1", [128, TOT, WD], dt.float8e4, kind="ExternalInput")
    disc = nc.dram_tensor("disc", [128, NG], dt.float32, kind="ExternalInput")
    w12 = nc.dram_tensor("w12", [128, KB, H], dt.bfloat16, kind="ExternalInput")
    out = nc.dram_tensor("out", [128, NG, H], dt.bfloat16, kind="ExternalOutput")

    with tile.TileContext(nc) as tc:
        with (
            tc.tile_pool(name="const", bufs=1) as cp,
            tc.tile_pool(name="io", bufs=3) as iop,
            tc.tile_pool(name="stg", bufs=2) as stgp,
            tc.tile_pool(name="ps", bufs=2, space="PSUM") as psp,
        ):
            wins = {}
            HW = LWIN // 2

            def load_window(nx):
                tt = iop.tile([128, LWIN, H], dt.bfloat16, tag="t2", bufs=4)
                nc.sync.dma_start(tt[:, :HW],
                                  t2[:, nx * LWIN:nx * LWIN + HW, :])
                nc.scalar.dma_start(tt[:, HW:],
                                    t2[:, nx * LWIN + HW:(nx + 1) * LWIN, :])
                ss = iop.tile([128, LWIN, WD], dt.float8e4, tag="s1", bufs=4)
                nc.gpsimd.dma_start(ss[:], s1[:, nx * LWIN:(nx + 1) * LWIN, :])
                wins[nx] = (tt, ss)
                wins.pop(nx - 4, None)
                ensure.hi = nx

            def ensure(wi):
                while ensure.hi < min(wi + 2, NW - 1) or ensure.hi < wi:
                    load_window(ensure.hi + 1)
            ensure.hi = -1

            load_window(0)
            w12_sb = cp.tile([128, KB, H], dt.bfloat16)
            nc.sync.dma_start(w12_sb[:], w12[:])
            disc_sb = cp.tile([128, NG], dt.float32)
            nc.scalar.dma_start(disc_sb[:], disc[:])

            def emit_psC(j):
                wi = (j * ST) // LWIN
                ensure(wi)
                tt, ss = wins[wi]
                psC = psp.tile([128, H], dt.float32, tag="psC", bufs=3)
                for st in range(ST):
                    sb = (j * ST) % LWIN + st
                    for f in range(KB):
                        o = f * 128 + st * WD
                        nc.tensor.matmul(psC[:, o:o + WD],
                                         tt[:, sb, f * 128:(f + 1) * 128],
                                         ss[:, sb, :], start=True, stop=True)
                zf = iop.tile([128, H], dt.bfloat16, tag="zf", bufs=3)
                if j % 2 == 0:
                    nc.vector.tensor_copy(zf[:], psC[:])
                else:
                    nc.scalar.activation(zf[:], psC[:], AF.Copy)
                return zf

            def emit_psD(j, zf, ows):
                psD = psp.tile([128, H], dt.float32, tag="psD", bufs=3)
                for k in range(KB):
                    nc.tensor.matmul(psD[:], zf[:, k * 128:(k + 1) * 128],
                                     w12_sb[:, k, :], start=(k == 0),
                                     stop=(k == KB - 1))
                if j % 2 == 0:
                    nc.scalar.activation(ows[:, j % SG, :], psD[:], AF.Copy,
                                         scale=disc_sb[:, j:j + 1])
                else:
                    nc.vector.tensor_scalar_mul(ows[:, j % SG, :], psD[:],
                                                disc_sb[:, j:j + 1])

            prev = None
            ows = None
            for j in range(NG + 1):
                zf = emit_psC(j) if j < NG else None
                if prev is not None:
                    pj, pzf = prev
                    if pj % SG == 0:
                        ows = stgp.tile([128, SG, H], dt.bfloat16, tag="ows")
                    emit_psD(pj, pzf, ows)
                    if pj % SG == SG - 1:
                        eng = nc.sync if (pj // SG) % 2 == 0 else nc.scalar
                        eng.dma_start(out[:, pj - SG + 1:pj + 1, :], ows[:])
                prev = (j, zf) if j < NG else None
    nc.compile()
    return nc


_CACHE = {}


def kernel(x, edge_index, W1, b1, W2, b2):
    from concourse import bass_utils

    in_maps, dims, perms, corr = _prep(x, edge_index, W1, b1, W2, b2)
    key = dims["NG"]
    if key not in _CACHE:
        _CACHE[key] = _build(dims)
    nc = _CACHE[key]
    res = bass_utils.run_bass_kernel_spmd(nc, in_maps, core_ids=list(range(M_CORES)))
    NG = dims["NG"]
    out = np.empty((N_NODES, H), np.float32)
    for cc in range(M_CORES):
        flat = np.asarray(res.results[cc]["out"]).reshape(128 * NG, H)
        out[cc * NPC:(cc + 1) * NPC] = flat[perms[cc]].astype(np.float32)
    out += corr
    return out


# revision 20
# speedup vs baseline: 2.1624x; 1.0223x over previous
"""GCN 2-layer message-passing block on 8 Trainium2 NeuronCores — v4.

Collapsed algebra: z = (S^2 x) W12^T + c2 vv^T + cvec b2^T with
S = D^-1/2 A D^-1/2, W12 = W2 W1, vv = W2 b1.  The host expands the
two-hop pairs (v, w) with merged path weights sum_u dis[u]^2, pre-scales
each token row xt[w] = dis[w] x[w] by its pair weight, and lays tokens
out in sub-tiles of <=32 destinations and <=128 tokens.  The device does
a single one-hot aggregation (3 matmuls of 32 cols per sub-tile, K~126)
straight into the projection layout [feat-in-block, dest], then projects
with W12 per 128-dest group (4 sub-tiles) and adds the rank-2 bias
correction via a K=2 matmul.  One aggregation pass, no intermediate
DRAM tensors: ~53MB HBM traffic and ~200k PE columns per core.
"""
import sys

sys.path.insert(0, "/opt/trn_rl_repo")

import numpy as np
import ml_dtypes

BF16 = ml_dtypes.bfloat16
FP8 = ml_dtypes.float8_e4m3

N_NODES = 100000
N_EDGES = 200000
H = 384
KB = H // 128
M_CORES = 8
NPC = N_NODES // M_CORES
WD = 32               # dests per sub-tile (one-hot width)
ST = 4                # sub-tiles per group (group = 128 dests)
SG = 4                # groups per output DMA
LWIN = 16             # sub-blocks per DMA window (= 4 groups)


def _pack2(sizes, captok=128, capcnt=WD):
    """Greedy one-bin-at-a-time pack of (nonzero-size) dests into
    sub-tiles with <=captok tokens and <=capcnt dests: fill each bin with
    the largest remaining size that fits.  Returns (sub_of, nb)."""
    import heapq
    order = np.argsort(-sizes, kind="stable")
    nb = max(int(np.ceil(sizes.sum() / captok)), 1)
    heap = [(0, 0, b) for b in range(nb)]
    heapq.heapify(heap)
    sub_of = np.empty(sizes.size, np.int64)
    for i in order:
        s = int(sizes[i])
        parked = []
        while True:
            if not heap:
                tok, cnt, b = 0, 0, nb
                nb += 1
                break
            tok, cnt, b = heapq.heappop(heap)
            if tok + s <= captok and cnt < capcnt:
                break
            parked.append((tok, cnt, b))
        for p in parked:
            heapq.heappush(heap, p)
        sub_of[i] = b
        tok += s
        cnt += 1
        if tok < captok and cnt < capcnt:
            heapq.heappush(heap, (tok, cnt, b))
    return sub_of, nb


def _prep(x, edge_index, W1, b1, W2, b2):
    row = np.asarray(edge_index[0], dtype=np.int64)
    col = np.asarray(edge_index[1], dtype=np.int64)
    xf = np.asarray(x, dtype=np.float64)

    deg = np.bincount(row, minlength=N_NODES).astype(np.float64)
    dis = deg ** -0.5
    a = np.bincount(col, weights=dis[row], minlength=N_NODES)
    cvec = dis * a
    c2 = dis * np.bincount(col, weights=(dis * cvec)[row], minlength=N_NODES)
    W12 = np.asarray(W2, np.float64) @ np.asarray(W1, np.float64)
    vv = np.asarray(W2, np.float64) @ np.asarray(b1, np.float64)
    xt = (dis[:, None] * xf).astype(np.float32)

    # two-hop pairs: for each edge e=(u,v), expand in-edges (w,u)
    indeg = np.bincount(col, minlength=N_NODES)
    order_c = np.argsort(col, kind="stable")
    row_by_col = row[order_c]
    coff = np.zeros(N_NODES + 1, np.int64)
    np.cumsum(indeg, out=coff[1:])

    cnts = indeg[row]
    P = int(cnts.sum())
    csum = np.zeros(N_EDGES + 1, np.int64)
    np.cumsum(cnts, out=csum[1:])
    eidx = np.repeat(np.arange(N_EDGES), cnts)
    rank = np.arange(P) - np.repeat(csum[:-1], cnts)
    w_pair = row_by_col[np.repeat(coff[row], cnts) + rank]
    v_pair = col[eidx]
    wt_pair = (dis[row] ** 2)[eidx]
    key = v_pair * N_NODES + w_pair
    ukey, inv = np.unique(key, return_inverse=True)
    wts = np.bincount(inv, weights=wt_pair)
    v_m = ukey // N_NODES
    w_m = ukey % N_NODES

    w12 = np.ascontiguousarray(
        W12.T.astype(BF16).reshape(KB, 128, H).transpose(1, 0, 2))
    vb2 = np.stack([vv.astype(BF16),
                    np.asarray(b2, np.float64).astype(BF16)], axis=0)

    # per-core packing (shared program => shared NG)
    cores = []
    for cc in range(M_CORES):
        m = (v_m // NPC) == cc
        dlo = v_m[m] - cc * NPC
        wsrc = w_m[m]
        wt = wts[m]
        sizes = np.bincount(dlo, minlength=NPC)
        # pack nonzero dests token-tight, then fill count slots with zeros
        nz = np.nonzero(sizes > 0)[0]
        sub_nz, nb = _pack2(sizes[nz])
        sub_of = np.empty(NPC, np.int64)
        sub_of[nz] = sub_nz
        zr = np.nonzero(sizes == 0)[0]
        free = WD - np.bincount(sub_nz, minlength=nb)
        slots = np.repeat(np.arange(nb), free)
        if slots.size < zr.size:
            extra = -(-(zr.size - slots.size) // WD)
            slots = np.concatenate(
                [slots, np.repeat(np.arange(nb, nb + extra), WD)])
            nb += extra
        sub_of[zr] = slots[:zr.size]
        cores.append((dlo, wsrc, wt, sizes, sub_of, nb))
    NG = max(-(-c[5] // ST) for c in cores)
    NG += (-NG) % (LWIN // ST)
    TOT = NG * ST

    in_maps = []
    perms = []
    for cc, (dlo, wsrc, wt, sizes, sub_of, nb) in enumerate(cores):
        # dest slot + token offset within sub-tile
        order = np.lexsort((np.arange(NPC), sub_of))
        dslot = np.empty(NPC, np.int64)
        tok_off = np.empty(NPC, np.int64)
        cnt = np.zeros(nb, np.int64)
        tok = np.zeros(nb, np.int64)
        for i in order:
            t = sub_of[i]
            dslot[i] = cnt[t]
            tok_off[i] = tok[t]
            cnt[t] += 1
            tok[t] += sizes[i]

        # pairs sorted by (dlo, w) already (ukey order)
        starts = np.zeros(NPC + 1, np.int64)
        np.cumsum(sizes, out=starts[1:])
        prank = np.arange(dlo.size) - starts[dlo]
        part = tok_off[dlo] + prank
        sb = sub_of[dlo]

        t2 = np.zeros((128, TOT, H), dtype=BF16)
        t2[part, sb] = (xt[wsrc] * wt[:, None].astype(np.float32))
        s1 = np.zeros((128, TOT, WD), dtype=FP8)
        s1[part, sb, dslot[dlo]] = 1.0

        g_d = sub_of // ST
        part_d = (sub_of % ST) * WD + dslot
        dd = dis[cc * NPC:(cc + 1) * NPC]
        disc = np.zeros((128, NG), dtype=np.float32)
        disc[part_d, g_d] = dd.astype(np.float32)

        in_maps.append({
            "t2": t2, "s1": s1, "disc": disc, "w12": w12,
        })
        perms.append(part_d * NG + g_d)
    # x-independent rank-2 bias, added host-side after the gather
    corr = (np.outer(c2, vv) + np.outer(cvec, np.asarray(b2, np.float64))
            ).astype(np.float32)
    return in_maps, dict(NG=NG), perms, corr


def _build(dims):
    import concourse.bass as bass
    import concourse.bacc as bacc
    import concourse.mybir as mybir
    import concourse.tile as tile

    dt = mybir.dt
    AF = mybir.ActivationFunctionType
    NG = dims["NG"]
    TOT = NG * ST
    NW = TOT // LWIN

    nc = bacc.Bacc(None, target_bir_lowering=False)
    t2 = nc.dram_tensor("t2", [128, TOT, H], dt.bfloat16, kind="ExternalInput")
    s1 = nc.dram_tensor("s1", [128, TOT, WD], dt.float8e4, kind="ExternalInput")
    disc = nc.dram_tensor("disc", [128, NG], dt.float32, kind="ExternalInput")
    w12 = nc.dram_tensor("w12", [128, KB, H], dt.bfloat16, kind="ExternalInput")
    out = nc.dram_tensor("out", [128, NG, H], dt.bfloat16, kind="ExternalOutput")

    with tile.TileContext(nc) as tc:
        with (
            tc.tile_pool(name="const", bufs=1) as cp,
            tc.tile_pool(name="io", bufs=3) as iop,
            tc.tile_pool(name="stg", bufs=2) as stgp,
            tc.tile_pool(name="ps", bufs=2, space="PSUM") as psp,
        ):
            wins = {}
            HW = LWIN // 2

            def load_window(nx):
                tt = iop.tile([128, LWIN, H], dt.bfloat16, tag="t2", bufs=4)
                nc.sync.dma_start(tt[:, :HW],
                                  t2[:, nx * LWIN:nx * LWIN + HW, :])
                nc.scalar.dma_start(tt[:, HW:],
                                    t2[:, nx * LWIN + HW:(nx + 1) * LWIN, :])
                ss = iop.tile([128, LWIN, WD], dt.float8e4, tag="s1", bufs=4)
                nc.sync.dma_start(ss[:, :HW], s1[:, nx * LWIN:nx * LWIN + HW, :])
                nc.scalar.dma_start(ss[:, HW:],
                                    s1[:, nx * LWIN + HW:(nx + 1) * LWIN, :])
                wins[nx] = (tt, ss)
                wins.pop(nx - 4, None)
                ensure.hi = nx

            def ensure(wi):
                while ensure.hi < min(wi + 2, NW - 1) or ensure.hi < wi:
                    load_window(ensure.hi + 1)
            ensure.hi = -1

            load_window(0)
            w12_sb = cp.tile([128, KB, H], dt.bfloat16)
            nc.sync.dma_start(w12_sb[:], w12[:])
            disc_sb = cp.tile([128, NG], dt.float32)
            nc.scalar.dma_start(disc_sb[:], disc[:])

            def emit_psC(j):
                wi = (j * ST) // LWIN
                ensure(wi)
                tt, ss = wins[wi]
                psC = psp.tile([128, H], dt.float32, tag="psC", bufs=4)
                for st in range(ST):
                    sb = (j * ST) % LWIN + st
                    for f in range(KB):
                        o = f * 128 + st * WD
                        nc.tensor.matmul(psC[:, o:o + WD],
                                         tt[:, sb, f * 128:(f + 1) * 128],
                                         ss[:, sb, :], start=True, stop=True)
                zf = iop.tile([128, H], dt.bfloat16, tag="zf", bufs=3)
                if j % 2 == 0:
                    nc.vector.tensor_copy(zf[:], psC[:])
                else:
                    nc.scalar.activation(zf[:], psC[:], AF.Copy)
                return zf

            def emit_psD(j, zf, ows):
                psD = psp.tile([128, H], dt.float32, tag="psD", bufs=3)
                for k in range(KB):
                    nc.tensor.matmul(psD[:], zf[:, k * 128:(k + 1) * 128],
                                     w12_sb[:, k, :], start=(k == 0),
                                     stop=(k == KB - 1))
                if j % 2 == 0:
                    nc.scalar.activation(ows[:, j % SG, :], psD[:], AF.Copy,
                                         scale=disc_sb[:, j:j + 1])
                else:
                    nc.vector.tensor_scalar_mul(ows[:, j % SG, :], psD[:],
                                                disc_sb[:, j:j + 1])

            prev = None
            ows = None
            for j in range(NG + 1):
                zf = emit_psC(j) if j < NG else None
                if prev is not None:
                    pj, pzf = prev
                    if pj % SG == 0:
                        ows = stgp.tile([128, SG, H], dt.bfloat16, tag="ows",
                                        bufs=4)
                    emit_psD(pj, pzf, ows)
                    if pj % SG == SG - 1:
                        nc.gpsimd.dma_start(
                            out[:, pj - SG + 1:pj + 1, :], ows[:])
                prev = (j, zf) if j < NG else None
    nc.compile()
    return nc


_CACHE = {}


def kernel(x, edge_index, W1, b1, W2, b2):
    from concourse import bass_utils

    in_maps, dims, perms, corr = _prep(x, edge_index, W1, b1, W2, b2)
    key = dims["NG"]
    if key not in _CACHE:
        _CACHE[key] = _build(dims)
    nc = _CACHE[key]
    res = bass_utils.run_bass_kernel_spmd(nc, in_maps, core_ids=list(range(M_CORES)))
    NG = dims["NG"]
    out = np.empty((N_NODES, H), np.float32)
    for cc in range(M_CORES):
        flat = np.asarray(res.results[cc]["out"]).reshape(128 * NG, H)
        out[cc * NPC:(cc + 1) * NPC] = flat[perms[cc]].astype(np.float32)
    out += corr
    return out


# revision 23
# speedup vs baseline: 2.2773x; 1.0531x over previous
"""GCN 2-layer message-passing block on 8 Trainium2 NeuronCores — v4.

Collapsed algebra: z = (S^2 x) W12^T + c2 vv^T + cvec b2^T with
S = D^-1/2 A D^-1/2, W12 = W2 W1, vv = W2 b1.  The host expands the
two-hop pairs (v, w) with merged path weights sum_u dis[u]^2, pre-scales
each token row xt[w] = dis[w] x[w] by its pair weight, and lays tokens
out in sub-tiles of <=32 destinations and <=128 tokens.  The device does
a single one-hot aggregation (3 matmuls of 32 cols per sub-tile, K~126)
straight into the projection layout [feat-in-block, dest], then projects
with W12 per 128-dest group (4 sub-tiles) and adds the rank-2 bias
correction via a K=2 matmul.  One aggregation pass, no intermediate
DRAM tensors: ~53MB HBM traffic and ~200k PE columns per core.
"""
import sys

sys.path.insert(0, "/opt/trn_rl_repo")

import numpy as np
import ml_dtypes

BF16 = ml_dtypes.bfloat16
FP8 = ml_dtypes.float8_e4m3

N_NODES = 100000
N_EDGES = 200000
H = 384
KB = H // 128
M_CORES = 8
NPC = N_NODES // M_CORES
WD = 32               # dests per sub-tile (one-hot width)
ST = 4                # sub-tiles per group (group = 128 dests)
SG = 4                # groups per output DMA
LWIN = 16             # sub-blocks per DMA window (= 4 groups)


def _pack2(sizes, captok=128, capcnt=WD):
    """Greedy one-bin-at-a-time pack of (nonzero-size) dests into
    sub-tiles with <=captok tokens and <=capcnt dests: fill each bin with
    the largest remaining size that fits.  Returns (sub_of, nb)."""
    import heapq
    order = np.argsort(-sizes, kind="stable")
    nb = max(int(np.ceil(sizes.sum() / captok)), 1)
    heap = [(0, 0, b) for b in range(nb)]
    heapq.heapify(heap)
    sub_of = np.empty(sizes.size, np.int64)
    for i in order:
        s = int(sizes[i])
        parked = []
        while True:
            if not heap:
                tok, cnt, b = 0, 0, nb
                nb += 1
                break
            tok, cnt, b = heapq.heappop(heap)
            if tok + s <= captok and cnt < capcnt:
                break
            parked.append((tok, cnt, b))
        for p in parked:
            heapq.heappush(heap, p)
        sub_of[i] = b
        tok += s
        cnt += 1
        if tok < captok and cnt < capcnt:
            heapq.heappush(heap, (tok, cnt, b))
    return sub_of, nb


def _prep(x, edge_index, W1, b1, W2, b2):
    row = np.asarray(edge_index[0], dtype=np.int64)
    col = np.asarray(edge_index[1], dtype=np.int64)
    xf = np.asarray(x, dtype=np.float64)

    deg = np.bincount(row, minlength=N_NODES).astype(np.float64)
    dis = deg ** -0.5
    a = np.bincount(col, weights=dis[row], minlength=N_NODES)
    cvec = dis * a
    c2 = dis * np.bincount(col, weights=(dis * cvec)[row], minlength=N_NODES)
    W12 = np.asarray(W2, np.float64) @ np.asarray(W1, np.float64)
    vv = np.asarray(W2, np.float64) @ np.asarray(b1, np.float64)
    xt = (dis[:, None] * xf).astype(np.float32)

    # two-hop pairs: for each edge e=(u,v), expand in-edges (w,u)
    indeg = np.bincount(col, minlength=N_NODES)
    order_c = np.argsort(col, kind="stable")
    row_by_col = row[order_c]
    coff = np.zeros(N_NODES + 1, np.int64)
    np.cumsum(indeg, out=coff[1:])

    cnts = indeg[row]
    P = int(cnts.sum())
    csum = np.zeros(N_EDGES + 1, np.int64)
    np.cumsum(cnts, out=csum[1:])
    eidx = np.repeat(np.arange(N_EDGES), cnts)
    rank = np.arange(P) - np.repeat(csum[:-1], cnts)
    w_pair = row_by_col[np.repeat(coff[row], cnts) + rank]
    v_pair = col[eidx]
    wt_pair = (dis[row] ** 2)[eidx]
    key = v_pair * N_NODES + w_pair
    ukey, inv = np.unique(key, return_inverse=True)
    wts = np.bincount(inv, weights=wt_pair)
    v_m = ukey // N_NODES
    w_m = ukey % N_NODES

    w12 = np.ascontiguousarray(
        W12.T.astype(BF16).reshape(KB, 128, H).transpose(1, 0, 2))
    vb2 = np.stack([vv.astype(BF16),
                    np.asarray(b2, np.float64).astype(BF16)], axis=0)

    # per-core packing (shared program => shared NG)
    cores = []
    for cc in range(M_CORES):
        m = (v_m // NPC) == cc
        dlo = v_m[m] - cc * NPC
        wsrc = w_m[m]
        wt = wts[m]
        sizes = np.bincount(dlo, minlength=NPC)
        # pack nonzero dests token-tight, then fill count slots with zeros
        nz = np.nonzero(sizes > 0)[0]
        sub_nz, nb = _pack2(sizes[nz])
        sub_of = np.empty(NPC, np.int64)
        sub_of[nz] = sub_nz
        zr = np.nonzero(sizes == 0)[0]
        free = WD - np.bincount(sub_nz, minlength=nb)
        slots = np.repeat(np.arange(nb), free)
        if slots.size < zr.size:
            extra = -(-(zr.size - slots.size) // WD)
            slots = np.concatenate(
                [slots, np.repeat(np.arange(nb, nb + extra), WD)])
            nb += extra
        sub_of[zr] = slots[:zr.size]
        cores.append((dlo, wsrc, wt, sizes, sub_of, nb))
    NG = max(-(-c[5] // ST) for c in cores)
    NG += (-NG) % (LWIN // ST)
    TOT = NG * ST

    in_maps = []
    perms = []
    for cc, (dlo, wsrc, wt, sizes, sub_of, nb) in enumerate(cores):
        # dest slot + token offset within sub-tile
        order = np.lexsort((np.arange(NPC), sub_of))
        dslot = np.empty(NPC, np.int64)
        tok_off = np.empty(NPC, np.int64)
        cnt = np.zeros(nb, np.int64)
        tok = np.zeros(nb, np.int64)
        for i in order:
            t = sub_of[i]
            dslot[i] = cnt[t]
            tok_off[i] = tok[t]
            cnt[t] += 1
            tok[t] += sizes[i]

        # pairs sorted by (dlo, w) already (ukey order)
        starts = np.zeros(NPC + 1, np.int64)
        np.cumsum(sizes, out=starts[1:])
        prank = np.arange(dlo.size) - starts[dlo]
        part = tok_off[dlo] + prank
        sb = sub_of[dlo]

        t2 = np.zeros((128, TOT, H), dtype=BF16)
        t2[part, sb] = (xt[wsrc] * wt[:, None].astype(np.float32))
        s1 = np.zeros((128, TOT, WD), dtype=FP8)
        s1[part, sb, dslot[dlo]] = 1.0

        g_d = sub_of // ST
        part_d = (sub_of % ST) * WD + dslot
        dd = dis[cc * NPC:(cc + 1) * NPC]
        disc = np.zeros((128, NG), dtype=np.float32)
        disc[part_d, g_d] = dd.astype(np.float32)

        in_maps.append({
            "t2": t2, "s1": s1, "disc": disc, "w12": w12,
        })
        perms.append(part_d * NG + g_d)
    # x-independent rank-2 bias, added host-side after the gather
    corr = (np.outer(c2, vv) + np.outer(cvec, np.asarray(b2, np.float64))
            ).astype(np.float32)
    return in_maps, dict(NG=NG), perms, corr


def _build(dims):
    import concourse.bass as bass
    import concourse.bacc as bacc
    import concourse.mybir as mybir
    import concourse.tile as tile

    dt = mybir.dt
    AF = mybir.ActivationFunctionType
    NG = dims["NG"]
    TOT = NG * ST
    NW = TOT // LWIN

    nc = bacc.Bacc(None, target_bir_lowering=False)
    t2 = nc.dram_tensor("t2", [128, TOT, H], dt.bfloat16, kind="ExternalInput")
    s1 = nc.dram_tensor("s1", [128, TOT, WD], dt.float8e4, kind="ExternalInput")
    disc = nc.dram_tensor("disc", [128, NG], dt.float32, kind="ExternalInput")
    w12 = nc.dram_tensor("w12", [128, KB, H], dt.bfloat16, kind="ExternalInput")
    out = nc.dram_tensor("out", [128, NG, H], dt.bfloat16, kind="ExternalOutput")

    with tile.TileContext(nc) as tc:
        with (
            tc.tile_pool(name="const", bufs=1) as cp,
            tc.tile_pool(name="io", bufs=3) as iop,
            tc.tile_pool(name="stg", bufs=2) as stgp,
            tc.tile_pool(name="ps", bufs=2, space="PSUM") as psp,
        ):
            wins = {}
            HW = LWIN // 2

            def load_window(nx):
                tt = iop.tile([128, LWIN, H], dt.bfloat16, tag="t2", bufs=4)
                if nx == 0:
                    qw = HW // 2
                    nc.sync.dma_start(tt[:, :qw], t2[:, :qw, :])
                    nc.sync.dma_start(tt[:, qw:HW], t2[:, qw:HW, :])
                else:
                    nc.sync.dma_start(tt[:, :HW],
                                      t2[:, nx * LWIN:nx * LWIN + HW, :])
                nc.scalar.dma_start(tt[:, HW:],
                                    t2[:, nx * LWIN + HW:(nx + 1) * LWIN, :])
                ss = iop.tile([128, LWIN, WD], dt.float8e4, tag="s1", bufs=4)
                nc.sync.dma_start(ss[:, :HW], s1[:, nx * LWIN:nx * LWIN + HW, :])
                nc.scalar.dma_start(ss[:, HW:],
                                    s1[:, nx * LWIN + HW:(nx + 1) * LWIN, :])
                wins[nx] = (tt, ss)
                wins.pop(nx - 4, None)
                ensure.hi = nx

            def ensure(wi):
                while ensure.hi < min(wi + 2, NW - 1) or ensure.hi < wi:
                    load_window(ensure.hi + 1)
            ensure.hi = -1

            load_window(0)
            w12_sb = cp.tile([128, KB, H], dt.bfloat16)
            nc.sync.dma_start(w12_sb[:], w12[:])
            disc_sb = cp.tile([128, NG], dt.float32)
            nc.scalar.dma_start(disc_sb[:], disc[:])

            def emit_psC(j):
                wi = (j * ST) // LWIN
                ensure(wi)
                tt, ss = wins[wi]
                psC = psp.tile([128, H], dt.float32, tag="psC", bufs=4)
                for st in range(ST):
                    sb = (j * ST) % LWIN + st
                    for f in range(KB):
                        o = f * 128 + st * WD
                        nc.tensor.matmul(psC[:, o:o + WD],
                                         tt[:, sb, f * 128:(f + 1) * 128],
                                         ss[:, sb, :], start=True, stop=True)
                zf = iop.tile([128, H], dt.bfloat16, tag="zf", bufs=4)
                if j % 2 == 0:
                    nc.vector.tensor_copy(zf[:], psC[:])
                else:
                    nc.scalar.activation(zf[:], psC[:], AF.Copy)
                return zf

            def emit_psD(j, zf, ows):
                psD = psp.tile([128, H], dt.float32, tag="psD", bufs=3)
                for k in range(KB):
                    nc.tensor.matmul(psD[:], zf[:, k * 128:(k + 1) * 128],
                                     w12_sb[:, k, :], start=(k == 0),
                                     stop=(k == KB - 1))
                if j % 2 == 0:
                    nc.scalar.activation(ows[:, j % SG, :], psD[:], AF.Copy,
                                         scale=disc_sb[:, j:j + 1])
                else:
                    nc.vector.tensor_scalar_mul(ows[:, j % SG, :], psD[:],
                                                disc_sb[:, j:j + 1])

            from collections import deque
            pend = deque()
            ows = None
            for j in range(NG + 2):
                if j < NG:
                    pend.append((j, emit_psC(j)))
                if len(pend) > 2 or (j >= NG and pend):
                    pj, pzf = pend.popleft()
                    if pj % SG == 0:
                        ows = stgp.tile([128, SG, H], dt.bfloat16, tag="ows",
                                        bufs=4)
                    emit_psD(pj, pzf, ows)
                    if pj % SG == SG - 1:
                        if (pj // SG) >= NG // SG - 2:
                            eng = nc.sync if (pj // SG) % 2 == 0 else nc.scalar
                        else:
                            eng = nc.gpsimd
                        eng.dma_start(out[:, pj - SG + 1:pj + 1, :], ows[:])
    nc.compile()
    return nc


_CACHE = {}


def kernel(x, edge_index, W1, b1, W2, b2):
    from concourse import bass_utils

    in_maps, dims, perms, corr = _prep(x, edge_index, W1, b1, W2, b2)
    key = dims["NG"]
    if key not in _CACHE:
        _CACHE[key] = _build(dims)
    nc = _CACHE[key]
    res = bass_utils.run_bass_kernel_spmd(nc, in_maps, core_ids=list(range(M_CORES)))
    NG = dims["NG"]
    out = np.empty((N_NODES, H), np.float32)
    for cc in range(M_CORES):
        flat = np.asarray(res.results[cc]["out"]).reshape(128 * NG, H)
        out[cc * NPC:(cc + 1) * NPC] = flat[perms[cc]].astype(np.float32)
    out += corr
    return out
